# revision 5
# baseline (speedup 1.0000x reference)
"""Trainium2 Bass kernel for Gaussian KDE evaluation.

reference math:
    val[m] = (1/N) * sum_n exp(t1 - 0.5*d2(m,n)/bw^2)
    d2(m,n) = |e_m|^2 + |b_n|^2 - 2<e_m, b_n>
    t1 = -0.5*D*log(2*pi) - log_bw,  bw^2 = exp(2*log_bw)

Strategy (8 NeuronCores, x_eval row-sharded, x_base/log_bw replicated):
  All operands are pre-scaled on host by s = sqrt(0.5)/bw so a single
  K=54 bf16 matmul produces the full negated exponent argument in PSUM:
      PSUM(m,n) = |b'|^2 - 2<e',b'> - bias_m,
      bias_m = t1 - ln(N) - |e'_m|^2,   tau = -PSUM.
  The cross term runs at full PE rate (1 col/cycle) in bf16 using a
  3-term hi/lo split (eh*bh + el*bh + eh*bl); |b'|^2 rides as three
  bf16 rows against ones in lhsT; -bias_m rides as three bf16 rows of
  lhsT against ones in rhs.  ScalarE does exp via ACTIVATE(scale=-1)
  over 2048-col PSUM blocks (4 banks, ping-ponged against the matmuls)
  with accum_out producing the row sums; one DVE tensor_reduce and a
  32B DMA finish.  The first two blocks are 512/1536 cols so compute
  starts as soon as the first small DMA lands.
"""

import numpy as np
import ml_dtypes

M, N, D = 8192, 16384, 16
NCORES = 8
MS = M // NCORES          # eval rows per core (1024)
RT = MS // 128            # row tiles per core (8)
K = 3 * D + 6             # 54: 3x16 cross + 3 |b'|^2 rows + 3 bias rows
CH = 2048                 # columns per exp/accum block (4 PSUM banks)
NCH = N // CH             # 8 blocks per row tile
SUMW = NCH + 1            # sums columns per row tile (9; rt0 uses all)
LOG_2PI = float(np.log(2.0 * np.pi))
BF16 = ml_dtypes.bfloat16

_CACHE = {}


def _blocks(rt):
    if rt == 0:
        return [(0, 512), (512, 1536)] + [(c, CH) for c in range(CH, N, CH)]
    return [(c, CH) for c in range(0, N, CH)]


# rhs pieces split across the two HWDGE queues (SP + Activation) so
# early pieces land ahead of the ScalarE consumption front.
_SY_PIECES = [(0, 512), (512, 1536), (2048, 2048), (6144, 2048),
              (10240, 2048), (14336, 2048)]
_ACT_PIECES = [(4096, 2048), (8192, 2048), (12288, 2048)]


def _build_nc():
    from concourse import bacc, mybir, tile

    f32 = mybir.dt.float32
    bf16 = mybir.dt.bfloat16
    nc = bacc.Bacc("TRN2", target_bir_lowering=False, debug=False,
                   num_devices=NCORES)

    lhsT = nc.dram_tensor("lhsT", [K, MS], bf16, kind="ExternalInput")
    rhs = nc.dram_tensor("rhs", [K, N], bf16, kind="ExternalInput")
    out = nc.dram_tensor("out", [128, RT], f32, kind="ExternalOutput")

    Exp = mybir.ActivationFunctionType.Exp
    ADD = mybir.AluOpType.add
    X = mybir.AxisListType.X

    with tile.TileContext(nc) as tc:
        with (
            tc.tile_pool(name="persist", bufs=1) as pp,
            tc.tile_pool(name="mm", bufs=2, space="PSUM") as mmp,
        ):
            rhs_sb = pp.tile([K, N], bf16)
            for c0, w in _ACT_PIECES:
                nc.scalar.dma_start(out=rhs_sb[:, c0:c0 + w],
                                    in_=rhs[:, c0:c0 + w])

            # Warm the exp table while DMAs are in flight.
            dummy = pp.tile([1, 1], f32)
            nc.vector.memset(dummy[:], 0.0)
            nc.scalar.activation(dummy[:], dummy[:], Exp)

            lhsT_sb = pp.tile([K, MS], bf16)
            nc.sync.dma_start(out=lhsT_sb[:], in_=lhsT[:])
            for c0, w in _SY_PIECES:
                nc.sync.dma_start(out=rhs_sb[:, c0:c0 + w],
                                  in_=rhs[:, c0:c0 + w])

            sums = pp.tile([128, RT * SUMW], f32)
            nc.vector.memset(sums[:], 0.0)

            for rt in range(RT):
                for bi, (c0, w) in enumerate(_blocks(rt)):
                    ps = mmp.tile([128, CH], f32, tag="mm")
                    for j in range(w // 512):
                        nc.tensor.matmul(
                            ps[:, j * 512:(j + 1) * 512],
                            lhsT_sb[:, rt * 128:(rt + 1) * 128],
                            rhs_sb[:, c0 + j * 512:c0 + (j + 1) * 512],
                            start=True, stop=True)
                    sc = rt * SUMW + bi
                    nc.scalar.activation(
                        ps[:, 0:w], ps[:, 0:w], Exp, scale=-1.0,
                        accum_out=sums[:, sc:sc + 1])

            val = pp.tile([128, RT], f32)
            nc.vector.tensor_reduce(
                out=val[:],
                in_=sums[:].rearrange("p (r c) -> p r c", c=SUMW),
                axis=X, op=ADD)
            nc.sync.dma_start(out=out[:], in_=val[:])

    nc.compile()
    return nc


def _split3(v):
    """Split fp64 array into three bf16 parts summing to ~fp32 accuracy."""
    p0 = v.astype(BF16)
    r = v - p0.astype(np.float64)
    p1 = r.astype(BF16)
    p2 = (r - p1.astype(np.float64)).astype(BF16)
    return p0, p1, p2


def _prepare_in_maps(x_eval, x_base, log_bw):
    """Host-side operand packing (numpy): pre-scale, bf16 hi/lo split."""
    x_eval = np.ascontiguousarray(x_eval, dtype=np.float32)
    x_base = np.ascontiguousarray(x_base, dtype=np.float32)
    lb = float(np.asarray(log_bw, dtype=np.float32).reshape(-1)[0])

    s = np.sqrt(0.5 * np.exp(-2.0 * lb))
    t1 = -0.5 * D * LOG_2PI - lb

    b = (x_base.astype(np.float64) * s).astype(np.float32)
    bh = b.astype(BF16)
    bl = (b - bh.astype(np.float32)).astype(BF16)
    s0, s1, s2 = _split3((b.astype(np.float64) ** 2).sum(1))
    rhs = np.empty((K, N), dtype=BF16)
    rhs[0:D] = bh.T
    rhs[D:2 * D] = bh.T
    rhs[2 * D:3 * D] = bl.T
    rhs[3 * D] = s0
    rhs[3 * D + 1] = s1
    rhs[3 * D + 2] = s2
    rhs[3 * D + 3:] = BF16(1.0)

    e = (x_eval.astype(np.float64) * s).astype(np.float32)
    eh = e.astype(BF16)
    el = (e - eh.astype(np.float32)).astype(BF16)
    sqe = (e.astype(np.float64) ** 2).sum(1)
    # PSUM carries -bias so tau = -PSUM; v = -bias = |e'|^2 + ln(N) - t1
    v0, v1, v2 = _split3(sqe + np.log(N) - t1)

    in_maps = []
    for i in range(NCORES):
        sl = slice(i * MS, (i + 1) * MS)
        lhsT = np.empty((K, MS), dtype=BF16)
        lhsT[0:D] = (-2.0 * eh[sl].astype(np.float32)).astype(BF16).T
        lhsT[D:2 * D] = (-2.0 * el[sl].astype(np.float32)).astype(BF16).T
        lhsT[2 * D:3 * D] = lhsT[0:D]
        lhsT[3 * D:3 * D + 3] = BF16(1.0)
        lhsT[3 * D + 3] = v0[sl]
        lhsT[3 * D + 4] = v1[sl]
        lhsT[3 * D + 5] = v2[sl]
        in_maps.append({"lhsT": lhsT, "rhs": rhs})
    return in_maps


def _unshard(results):
    # out[p, rt] = val of shard row rt*128 + p
    shards = [np.asarray(r["out"]).T.reshape(-1) for r in results]
    return np.concatenate(shards).astype(np.float32)


def kernel(x_eval, x_base, log_bw):
    from concourse.bass_utils import run_bass_kernel_spmd

    if "nc" not in _CACHE:
        _CACHE["nc"] = _build_nc()
    nc = _CACHE["nc"]

    in_maps = _prepare_in_maps(x_eval, x_base, log_bw)
    res = run_bass_kernel_spmd(nc, in_maps, list(range(NCORES)))
    return _unshard(res.results)


# revision 9
# speedup vs baseline: 1.0157x; 1.0157x over previous
"""Trainium2 Bass kernel for Gaussian KDE evaluation.

reference math:
    val[m] = (1/N) * sum_n exp(t1 - 0.5*d2(m,n)/bw^2)
    d2(m,n) = |e_m|^2 + |b_n|^2 - 2<e_m, b_n>
    t1 = -0.5*D*log(2*pi) - log_bw,  bw^2 = exp(2*log_bw)

Strategy (8 NeuronCores, x_eval row-sharded, x_base/log_bw replicated):
  All operands are pre-scaled on host by s = sqrt(0.5)/bw so a single
  K=54 bf16 matmul produces the full negated exponent argument in PSUM:
      PSUM(m,n) = |b'|^2 - 2<e',b'> - bias_m,
      bias_m = t1 - ln(N) - |e'_m|^2,   tau = -PSUM.
  The cross term runs at full PE rate (1 col/cycle) in bf16 using a
  3-term hi/lo split (eh*bh + el*bh + eh*bl); |b'|^2 rides as three
  bf16 rows against ones in lhsT; -bias_m rides as three bf16 rows of
  lhsT against ones in rhs.  ScalarE does exp via ACTIVATE(scale=-1)
  over 2048-col PSUM blocks (4 banks, ping-ponged against the matmuls)
  with accum_out producing the row sums; one DVE tensor_reduce and a
  32B DMA finish.  The first two blocks are 512/1536 cols so compute
  starts as soon as the first small DMA lands.
"""

import numpy as np
import ml_dtypes

M, N, D = 8192, 16384, 16
NCORES = 8
MS = M // NCORES          # eval rows per core (1024)
RT = MS // 128            # row tiles per core (8)
K = 3 * D + 6             # 54: 3x16 cross + 3 |b'|^2 rows + 3 bias rows
CH = 2048                 # columns per exp/accum block (4 PSUM banks)
NCH = N // CH             # 8 blocks per row tile
SUMW = NCH + 1            # sums columns per row tile (9; rt0 uses all)
LOG_2PI = float(np.log(2.0 * np.pi))
BF16 = ml_dtypes.bfloat16

_CACHE = {}


def _blocks(rt):
    if rt == 0:
        return [(0, 512), (512, 1536)] + [(c, CH) for c in range(CH, N, CH)]
    return [(c, CH) for c in range(0, N, CH)]


_SY_PIECES = [(0, 512), (512, 1536), (2048, 4096), (6144, 4096),
              (10240, 4096), (14336, 2048)]


def _build_nc():
    from concourse import bacc, mybir, tile

    f32 = mybir.dt.float32
    bf16 = mybir.dt.bfloat16
    nc = bacc.Bacc("TRN2", target_bir_lowering=False, debug=False,
                   num_devices=NCORES)

    lhsT = nc.dram_tensor("lhsT", [K, MS], bf16, kind="ExternalInput")
    rhs = nc.dram_tensor("rhs", [K, N], bf16, kind="ExternalInput")
    out = nc.dram_tensor("out", [128, RT], f32, kind="ExternalOutput")

    Exp = mybir.ActivationFunctionType.Exp
    ADD = mybir.AluOpType.add
    X = mybir.AxisListType.X

    with tile.TileContext(nc) as tc:
        with (
            tc.tile_pool(name="persist", bufs=1) as pp,
            tc.tile_pool(name="mm", bufs=2, space="PSUM") as mmp,
        ):
            rhs_sb = pp.tile([K, N], bf16)
            # Warm the exp table while DMAs are in flight.
            dummy = pp.tile([1, 1], f32)
            nc.vector.memset(dummy[:], 0.0)
            nc.scalar.activation(dummy[:], dummy[:], Exp)

            lhsT_sb = pp.tile([K, MS], bf16)
            nc.sync.dma_start(out=lhsT_sb[:, 0:128], in_=lhsT[:, 0:128])
            for i, (c0, w) in enumerate(_SY_PIECES):
                nc.sync.dma_start(out=rhs_sb[:, c0:c0 + w],
                                  in_=rhs[:, c0:c0 + w])
                if i == 1:
                    # rest of lhsT after the first two rhs pieces
                    nc.sync.dma_start(out=lhsT_sb[:, 128:MS],
                                      in_=lhsT[:, 128:MS])

            sums = pp.tile([128, RT * SUMW], f32)
            nc.vector.memset(sums[:], 0.0)

            for rt in range(RT):
                for bi, (c0, w) in enumerate(_blocks(rt)):
                    ps = mmp.tile([128, CH], f32, tag="mm")
                    for j in range(w // 512):
                        nc.tensor.matmul(
                            ps[:, j * 512:(j + 1) * 512],
                            lhsT_sb[:, rt * 128:(rt + 1) * 128],
                            rhs_sb[:, c0 + j * 512:c0 + (j + 1) * 512],
                            start=True, stop=True)
                    sc = rt * SUMW + bi
                    nc.scalar.activation(
                        ps[:, 0:w], ps[:, 0:w], Exp, scale=-1.0,
                        accum_out=sums[:, sc:sc + 1])

            val = pp.tile([128, RT], f32)
            nc.vector.tensor_reduce(
                out=val[:],
                in_=sums[:].rearrange("p (r c) -> p r c", c=SUMW),
                axis=X, op=ADD)
            nc.sync.dma_start(out=out[:], in_=val[:])

    nc.compile()
    return nc


def _split3(v):
    """Split fp64 array into three bf16 parts summing to ~fp32 accuracy."""
    p0 = v.astype(BF16)
    r = v - p0.astype(np.float64)
    p1 = r.astype(BF16)
    p2 = (r - p1.astype(np.float64)).astype(BF16)
    return p0, p1, p2


def _prepare_in_maps(x_eval, x_base, log_bw):
    """Host-side operand packing (numpy): pre-scale, bf16 hi/lo split."""
    x_eval = np.ascontiguousarray(x_eval, dtype=np.float32)
    x_base = np.ascontiguousarray(x_base, dtype=np.float32)
    lb = float(np.asarray(log_bw, dtype=np.float32).reshape(-1)[0])

    s = np.sqrt(0.5 * np.exp(-2.0 * lb))
    t1 = -0.5 * D * LOG_2PI - lb

    b = (x_base.astype(np.float64) * s).astype(np.float32)
    bh = b.astype(BF16)
    bl = (b - bh.astype(np.float32)).astype(BF16)
    s0, s1, s2 = _split3((b.astype(np.float64) ** 2).sum(1))
    rhs = np.empty((K, N), dtype=BF16)
    rhs[0:D] = bh.T
    rhs[D:2 * D] = bh.T
    rhs[2 * D:3 * D] = bl.T
    rhs[3 * D] = s0
    rhs[3 * D + 1] = s1
    rhs[3 * D + 2] = s2
    rhs[3 * D + 3:] = BF16(1.0)

    e = (x_eval.astype(np.float64) * s).astype(np.float32)
    eh = e.astype(BF16)
    el = (e - eh.astype(np.float32)).astype(BF16)
    sqe = (e.astype(np.float64) ** 2).sum(1)
    # PSUM carries -bias so tau = -PSUM; v = -bias = |e'|^2 + ln(N) - t1
    v0, v1, v2 = _split3(sqe + np.log(N) - t1)

    in_maps = []
    for i in range(NCORES):
        sl = slice(i * MS, (i + 1) * MS)
        lhsT = np.empty((K, MS), dtype=BF16)
        lhsT[0:D] = (-2.0 * eh[sl].astype(np.float32)).astype(BF16).T
        lhsT[D:2 * D] = (-2.0 * el[sl].astype(np.float32)).astype(BF16).T
        lhsT[2 * D:3 * D] = lhsT[0:D]
        lhsT[3 * D:3 * D + 3] = BF16(1.0)
        lhsT[3 * D + 3] = v0[sl]
        lhsT[3 * D + 4] = v1[sl]
        lhsT[3 * D + 5] = v2[sl]
        in_maps.append({"lhsT": lhsT, "rhs": rhs})
    return in_maps


def _unshard(results):
    # out[p, rt] = val of shard row rt*128 + p
    shards = [np.asarray(r["out"]).T.reshape(-1) for r in results]
    return np.concatenate(shards).astype(np.float32)


def kernel(x_eval, x_base, log_bw):
    from concourse.bass_utils import run_bass_kernel_spmd

    if "nc" not in _CACHE:
        _CACHE["nc"] = _build_nc()
    nc = _CACHE["nc"]

    in_maps = _prepare_in_maps(x_eval, x_base, log_bw)
    res = run_bass_kernel_spmd(nc, in_maps, list(range(NCORES)))
    return _unshard(res.results)


# revision 10
# speedup vs baseline: 1.0188x; 1.0031x over previous
"""Trainium2 Bass kernel for Gaussian KDE evaluation.

reference math:
    val[m] = (1/N) * sum_n exp(t1 - 0.5*d2(m,n)/bw^2)
    d2(m,n) = |e_m|^2 + |b_n|^2 - 2<e_m, b_n>
    t1 = -0.5*D*log(2*pi) - log_bw,  bw^2 = exp(2*log_bw)

Strategy (8 NeuronCores, x_eval row-sharded, x_base/log_bw replicated):
  All operands are pre-scaled on host by s = sqrt(0.5)/bw so a single
  K=54 bf16 matmul produces the full negated exponent argument in PSUM:
      PSUM(m,n) = |b'|^2 - 2<e',b'> - bias_m,
      bias_m = t1 - ln(N) - |e'_m|^2,   tau = -PSUM.
  The cross term runs at full PE rate (1 col/cycle) in bf16 using a
  3-term hi/lo split (eh*bh + el*bh + eh*bl); |b'|^2 rides as three
  bf16 rows against ones in lhsT; -bias_m rides as three bf16 rows of
  lhsT against ones in rhs.  ScalarE does exp via ACTIVATE(scale=-1)
  over 2048-col PSUM blocks (4 banks, ping-ponged against the matmuls)
  with accum_out producing the row sums; one DVE tensor_reduce and a
  32B DMA finish.  The first two blocks are 512/1536 cols so compute
  starts as soon as the first small DMA lands.
"""

import numpy as np
import ml_dtypes

M, N, D = 8192, 16384, 16
NCORES = 8
MS = M // NCORES          # eval rows per core (1024)
RT = MS // 128            # row tiles per core (8)
K = 3 * D + 6             # 54: 3x16 cross + 3 |b'|^2 rows + 3 bias rows
CH = 2048                 # columns per exp/accum block (4 PSUM banks)
NCH = N // CH             # 8 blocks per row tile
SUMW = NCH + 1            # sums columns per row tile (9; rt0 uses all)
LOG_2PI = float(np.log(2.0 * np.pi))
BF16 = ml_dtypes.bfloat16

_CACHE = {}


def _blocks(rt):
    if rt == 0:
        return [(0, 512), (512, 1536)] + [(c, CH) for c in range(CH, N, CH)]
    return [(c, CH) for c in range(0, N, CH)]


_SY_PIECES = [(0, 512), (512, 1536), (2048, 4096), (6144, 4096),
              (10240, 4096), (14336, 2048)]


def _build_nc():
    from concourse import bacc, mybir, tile

    f32 = mybir.dt.float32
    bf16 = mybir.dt.bfloat16
    nc = bacc.Bacc("TRN2", target_bir_lowering=False, debug=False,
                   num_devices=NCORES)

    lhsT = nc.dram_tensor("lhsT", [K, MS], bf16, kind="ExternalInput")
    rhs = nc.dram_tensor("rhs", [K, N], bf16, kind="ExternalInput")
    out = nc.dram_tensor("out", [128, RT], f32, kind="ExternalOutput")

    Exp = mybir.ActivationFunctionType.Exp
    ADD = mybir.AluOpType.add
    X = mybir.AxisListType.X

    with tile.TileContext(nc) as tc:
        with (
            tc.tile_pool(name="persist", bufs=1) as pp,
            tc.tile_pool(name="mm", bufs=2, space="PSUM") as mmp,
        ):
            rhs_sb = pp.tile([K, N], bf16)
            # Warm the exp table while DMAs are in flight.
            dummy = pp.tile([1, 1], f32)
            nc.vector.memset(dummy[:], 0.0)
            nc.scalar.activation(dummy[:], dummy[:], Exp)

            lhsT_sb = pp.tile([K, MS], bf16)
            nc.sync.dma_start(out=lhsT_sb[:, 0:128], in_=lhsT[:, 0:128])
            for c0, w in _SY_PIECES:
                nc.sync.dma_start(out=rhs_sb[:, c0:c0 + w],
                                  in_=rhs[:, c0:c0 + w])
            # rest of lhsT; not needed until rt=1 (~27us in)
            nc.sync.dma_start(out=lhsT_sb[:, 128:MS],
                              in_=lhsT[:, 128:MS])

            sums = pp.tile([128, RT * SUMW], f32)
            nc.vector.memset(sums[:], 0.0)

            for rt in range(RT):
                for bi, (c0, w) in enumerate(_blocks(rt)):
                    ps = mmp.tile([128, CH], f32, tag="mm")
                    for j in range(w // 512):
                        nc.tensor.matmul(
                            ps[:, j * 512:(j + 1) * 512],
                            lhsT_sb[:, rt * 128:(rt + 1) * 128],
                            rhs_sb[:, c0 + j * 512:c0 + (j + 1) * 512],
                            start=True, stop=True)
                    sc = rt * SUMW + bi
                    nc.scalar.activation(
                        ps[:, 0:w], ps[:, 0:w], Exp, scale=-1.0,
                        accum_out=sums[:, sc:sc + 1])

            val = pp.tile([128, RT], f32)
            nc.vector.tensor_reduce(
                out=val[:],
                in_=sums[:].rearrange("p (r c) -> p r c", c=SUMW),
                axis=X, op=ADD)
            nc.sync.dma_start(out=out[:], in_=val[:])

    nc.compile()
    return nc


def _split3(v):
    """Split fp64 array into three bf16 parts summing to ~fp32 accuracy."""
    p0 = v.astype(BF16)
    r = v - p0.astype(np.float64)
    p1 = r.astype(BF16)
    p2 = (r - p1.astype(np.float64)).astype(BF16)
    return p0, p1, p2


def _prepare_in_maps(x_eval, x_base, log_bw):
    """Host-side operand packing (numpy): pre-scale, bf16 hi/lo split."""
    x_eval = np.ascontiguousarray(x_eval, dtype=np.float32)
    x_base = np.ascontiguousarray(x_base, dtype=np.float32)
    lb = float(np.asarray(log_bw, dtype=np.float32).reshape(-1)[0])

    s = np.sqrt(0.5 * np.exp(-2.0 * lb))
    t1 = -0.5 * D * LOG_2PI - lb

    b = (x_base.astype(np.float64) * s).astype(np.float32)
    bh = b.astype(BF16)
    bl = (b - bh.astype(np.float32)).astype(BF16)
    s0, s1, s2 = _split3((b.astype(np.float64) ** 2).sum(1))
    rhs = np.empty((K, N), dtype=BF16)
    rhs[0:D] = bh.T
    rhs[D:2 * D] = bh.T
    rhs[2 * D:3 * D] = bl.T
    rhs[3 * D] = s0
    rhs[3 * D + 1] = s1
    rhs[3 * D + 2] = s2
    rhs[3 * D + 3:] = BF16(1.0)

    e = (x_eval.astype(np.float64) * s).astype(np.float32)
    eh = e.astype(BF16)
    el = (e - eh.astype(np.float32)).astype(BF16)
    sqe = (e.astype(np.float64) ** 2).sum(1)
    # PSUM carries -bias so tau = -PSUM; v = -bias = |e'|^2 + ln(N) - t1
    v0, v1, v2 = _split3(sqe + np.log(N) - t1)

    in_maps = []
    for i in range(NCORES):
        sl = slice(i * MS, (i + 1) * MS)
        lhsT = np.empty((K, MS), dtype=BF16)
        lhsT[0:D] = (-2.0 * eh[sl].astype(np.float32)).astype(BF16).T
        lhsT[D:2 * D] = (-2.0 * el[sl].astype(np.float32)).astype(BF16).T
        lhsT[2 * D:3 * D] = lhsT[0:D]
        lhsT[3 * D:3 * D + 3] = BF16(1.0)
        lhsT[3 * D + 3] = v0[sl]
        lhsT[3 * D + 4] = v1[sl]
        lhsT[3 * D + 5] = v2[sl]
        in_maps.append({"lhsT": lhsT, "rhs": rhs})
    return in_maps


def _unshard(results):
    # out[p, rt] = val of shard row rt*128 + p
    shards = [np.asarray(r["out"]).T.reshape(-1) for r in results]
    return np.concatenate(shards).astype(np.float32)


def kernel(x_eval, x_base, log_bw):
    from concourse.bass_utils import run_bass_kernel_spmd

    if "nc" not in _CACHE:
        _CACHE["nc"] = _build_nc()
    nc = _CACHE["nc"]

    in_maps = _prepare_in_maps(x_eval, x_base, log_bw)
    res = run_bass_kernel_spmd(nc, in_maps, list(range(NCORES)))
    return _unshard(res.results)


# revision 13
# speedup vs baseline: 1.0254x; 1.0065x over previous
"""Trainium2 Bass kernel for Gaussian KDE evaluation.

reference math:
    val[m] = (1/N) * sum_n exp(t1 - 0.5*d2(m,n)/bw^2)
    d2(m,n) = |e_m|^2 + |b_n|^2 - 2<e_m, b_n>
    t1 = -0.5*D*log(2*pi) - log_bw,  bw^2 = exp(2*log_bw)

Strategy (8 NeuronCores, x_eval row-sharded, x_base/log_bw replicated):
  All operands are pre-scaled on host by s = sqrt(0.5)/bw so a single
  K=54 bf16 matmul produces the full negated exponent argument in PSUM:
      PSUM(m,n) = |b'|^2 - 2<e',b'> - bias_m,
      bias_m = t1 - ln(N) - |e'_m|^2,   tau = -PSUM.
  The cross term runs at full PE rate (1 col/cycle) in bf16 using a
  3-term hi/lo split (eh*bh + el*bh + eh*bl); |b'|^2 rides as three
  bf16 rows against ones in lhsT; -bias_m rides as three bf16 rows of
  lhsT against ones in rhs.  ScalarE does exp via ACTIVATE(scale=-1)
  over 2048-col PSUM blocks (4 banks, ping-ponged against the matmuls)
  with accum_out producing the row sums; one DVE tensor_reduce and a
  32B DMA finish.  The first two blocks are 512/1536 cols so compute
  starts as soon as the first small DMA lands.
"""

import numpy as np
import ml_dtypes

M, N, D = 8192, 16384, 16
NCORES = 8
MS = M // NCORES          # eval rows per core (1024)
RT = MS // 128            # row tiles per core (8)
K = 3 * D + 6             # 54: 3x16 cross + 3 |b'|^2 rows + 3 bias rows
CH = 2048                 # columns per exp/accum block (4 PSUM banks)
NCH = N // CH             # 8 blocks per row tile
SUMW = NCH + 1            # sums columns per row tile (9; rt0 uses all)
LOG_2PI = float(np.log(2.0 * np.pi))
BF16 = ml_dtypes.bfloat16

_CACHE = {}


def _block_seq():
    """(rt, c0, w) issue order: column-outer so early ACTs only need the
    first resident columns while later DMA pieces stream in; rt0's first
    2048 cols are split 512/1536 to prime the pipeline, with rt1's full
    block issued between them to cover the wait for the 1536-col piece."""
    seq = [(0, 0, 512), (1, 0, CH), (0, 512, 1536)]
    seq += [(rt, 0, CH) for rt in range(2, RT)]
    for c in range(CH, N, CH):
        seq += [(rt, c, CH) for rt in range(RT)]
    return seq


_SY_PIECES = [(512, 1536), (2048, 4096), (6144, 4096),
              (10240, 4096), (14336, 2048)]


def _build_nc():
    from concourse import bacc, mybir, tile

    f32 = mybir.dt.float32
    bf16 = mybir.dt.bfloat16
    nc = bacc.Bacc("TRN2", target_bir_lowering=False, debug=False,
                   num_devices=NCORES)

    lhsT = nc.dram_tensor("lhsT", [K, MS], bf16, kind="ExternalInput")
    rhs = nc.dram_tensor("rhs", [K, N], bf16, kind="ExternalInput")
    out = nc.dram_tensor("out", [128, RT], f32, kind="ExternalOutput")

    Exp = mybir.ActivationFunctionType.Exp
    ADD = mybir.AluOpType.add
    X = mybir.AxisListType.X

    with tile.TileContext(nc) as tc:
        with (
            tc.tile_pool(name="persist", bufs=1) as pp,
            tc.tile_pool(name="mm", bufs=2, space="PSUM") as mmp,
        ):
            rhs_sb = pp.tile([K, N], bf16)
            # Warm the exp table while DMAs are in flight.
            dummy = pp.tile([1, 1], f32)
            nc.vector.memset(dummy[:], 0.0)
            nc.scalar.activation(dummy[:], dummy[:], Exp)

            lhsT_sb = pp.tile([K, MS], bf16)
            nc.sync.dma_start(out=lhsT_sb[:, 0:128], in_=lhsT[:, 0:128])
            nc.sync.dma_start(out=rhs_sb[:, 0:512], in_=rhs[:, 0:512])
            nc.sync.dma_start(out=lhsT_sb[:, 128:MS], in_=lhsT[:, 128:MS])
            for c0, w in _SY_PIECES:
                nc.sync.dma_start(out=rhs_sb[:, c0:c0 + w],
                                  in_=rhs[:, c0:c0 + w])

            sums = pp.tile([128, RT * SUMW], f32)
            nc.vector.memset(sums[:], 0.0)

            nblk = [0] * RT
            for rt, c0, w in _block_seq():
                ps = mmp.tile([128, CH], f32, tag="mm")
                for j in range(w // 512):
                    nc.tensor.matmul(
                        ps[:, j * 512:(j + 1) * 512],
                        lhsT_sb[:, rt * 128:(rt + 1) * 128],
                        rhs_sb[:, c0 + j * 512:c0 + (j + 1) * 512],
                        start=True, stop=True)
                sc = rt * SUMW + nblk[rt]
                nblk[rt] += 1
                nc.scalar.activation(
                    ps[:, 0:w], ps[:, 0:w], Exp, scale=-1.0,
                    accum_out=sums[:, sc:sc + 1])

            val = pp.tile([128, RT], f32)
            nc.vector.tensor_reduce(
                out=val[:],
                in_=sums[:].rearrange("p (r c) -> p r c", c=SUMW),
                axis=X, op=ADD)
            nc.sync.dma_start(out=out[:], in_=val[:])

    nc.compile()
    return nc


def _split3(v):
    """Split fp64 array into three bf16 parts summing to ~fp32 accuracy."""
    p0 = v.astype(BF16)
    r = v - p0.astype(np.float64)
    p1 = r.astype(BF16)
    p2 = (r - p1.astype(np.float64)).astype(BF16)
    return p0, p1, p2


def _prepare_in_maps(x_eval, x_base, log_bw):
    """Host-side operand packing (numpy): pre-scale, bf16 hi/lo split."""
    x_eval = np.ascontiguousarray(x_eval, dtype=np.float32)
    x_base = np.ascontiguousarray(x_base, dtype=np.float32)
    lb = float(np.asarray(log_bw, dtype=np.float32).reshape(-1)[0])

    s = np.sqrt(0.5 * np.exp(-2.0 * lb))
    t1 = -0.5 * D * LOG_2PI - lb

    b = (x_base.astype(np.float64) * s).astype(np.float32)
    bh = b.astype(BF16)
    bl = (b - bh.astype(np.float32)).astype(BF16)
    s0, s1, s2 = _split3((b.astype(np.float64) ** 2).sum(1))
    rhs = np.empty((K, N), dtype=BF16)
    rhs[0:D] = bh.T
    rhs[D:2 * D] = bh.T
    rhs[2 * D:3 * D] = bl.T
    rhs[3 * D] = s0
    rhs[3 * D + 1] = s1
    rhs[3 * D + 2] = s2
    rhs[3 * D + 3:] = BF16(1.0)

    e = (x_eval.astype(np.float64) * s).astype(np.float32)
    eh = e.astype(BF16)
    el = (e - eh.astype(np.float32)).astype(BF16)
    sqe = (e.astype(np.float64) ** 2).sum(1)
    # PSUM carries -bias so tau = -PSUM; v = -bias = |e'|^2 + ln(N) - t1
    v0, v1, v2 = _split3(sqe + np.log(N) - t1)

    in_maps = []
    for i in range(NCORES):
        sl = slice(i * MS, (i + 1) * MS)
        lhsT = np.empty((K, MS), dtype=BF16)
        lhsT[0:D] = (-2.0 * eh[sl].astype(np.float32)).astype(BF16).T
        lhsT[D:2 * D] = (-2.0 * el[sl].astype(np.float32)).astype(BF16).T
        lhsT[2 * D:3 * D] = lhsT[0:D]
        lhsT[3 * D:3 * D + 3] = BF16(1.0)
        lhsT[3 * D + 3] = v0[sl]
        lhsT[3 * D + 4] = v1[sl]
        lhsT[3 * D + 5] = v2[sl]
        in_maps.append({"lhsT": lhsT, "rhs": rhs})
    return in_maps


def _unshard(results):
    # out[p, rt] = val of shard row rt*128 + p
    shards = [np.asarray(r["out"]).T.reshape(-1) for r in results]
    return np.concatenate(shards).astype(np.float32)


def kernel(x_eval, x_base, log_bw):
    from concourse.bass_utils import run_bass_kernel_spmd

    if "nc" not in _CACHE:
        _CACHE["nc"] = _build_nc()
    nc = _CACHE["nc"]

    in_maps = _prepare_in_maps(x_eval, x_base, log_bw)
    res = run_bass_kernel_spmd(nc, in_maps, list(range(NCORES)))
    return _unshard(res.results)
